# revision 37
# baseline (speedup 1.0000x reference)
"""Causal self-attention Trainium2 Bass kernel.

Problem: B=4, S=2048, C=1024, H=16 heads, D=64 head_dim.
  qkv = x @ qkv_w.T + qkv_b ; per-head causal softmax attention ; out = attn @ proj_w.T + proj_b

Sharding (8 cores): core = 2*b + hg  (data parallel over batch b=0..3,
tensor parallel over 2 head-groups of 8 heads).  Each core computes
q/k/v for its 8 heads over the full sequence, does causal attention
locally, and computes a partial output projection (contraction over its
512 channels).  Host sums the two partials per batch.

Device layout choices (all matmuls bf16 inputs, fp32 PSUM accumulate):
  - qkvT produced in transposed [c', s] orientation directly from the
    projection (lhsT=wT tile, rhs=xT tile), so per-head qT/kT tiles
    [d=64, s] are ready for the scores matmul with zero transposes.
  - scores computed transposed: sT[k,q] = kT.T @ qT (contraction d on
    partitions).  The two heads of a pair run as concurrent row-tiled
    matmuls (PE rows 0-63 / 64-127).  Softmax sums over k (partition
    dim) come free from a ones-column appended to v in the AV matmul.
    No max-subtraction (init scale 0.02 keeps |scores| small).
  - v produced in [s, c'] orientation (lhsT=xT tile, rhs=wv tile) which
    is exactly the AV lhsT layout.
  - causal masking: full 128x512 score blocks beyond the diagonal are
    skipped; diagonal blocks are stream-sliced to the valid q columns
    and masked after exp.
  - attention is software-pipelined: the AV matmuls of group g are
    emitted one group late so the PE never sits behind the exp (ACT) of
    its own group; independent stage-1/stage-3 matmul chunks are
    slotted between groups to cover the ACT latency.
"""

import numpy as np
import ml_dtypes

import concourse.bacc as bacc
import concourse.bass as bass
import concourse.mybir as mybir
import concourse.tile as tile
from concourse.bass_utils import run_bass_kernel_spmd

BF = ml_dtypes.bfloat16
F32 = mybir.dt.float32
BF16 = mybir.dt.bfloat16
EXP = mybir.ActivationFunctionType.Exp

B, S, C = 4, 2048, 1024
H, D = 16, 64
P = 128
NQ = 512            # q-chunk (psum bank free size)
NSQ = S // NQ       # 4 q-chunks
NKB = S // P        # 16 k-blocks
CO = C // P         # 8 contraction tiles for stage 1
CPH = 512           # channels per head-group (8 heads * 64)

LAST_RESULTS = None
_NC_CACHE = []


def _ensure_axon_hooks():
    """Provide antenv.axon_hooks (NTFF profile hook) when the image lacks it.

    concourse.bass_utils imports it unconditionally on the trace path; this
    container's antenv has no axon_hooks module, but the axon PJRT .so does
    export the profiling C ABI.  Recreates the slim ctypes hook from
    trn_boot._ntff_profile_via_ctypes.  Also stubs out the S3 artifact
    upload (no credentials in-container).
    """
    import sys
    import types
    import contextlib
    import ctypes
    import os

    from concourse import bass_utils as _bu
    _bu.upload_artifacts = lambda tmpdir: str(tmpdir)

    try:
        import antenv.axon_hooks  # noqa: F401
        return
    except ImportError:
        pass

    state = {}

    def set_axon_ntff_profile_hook(hook):
        state["hook"] = hook

    def get_axon_ntff_profile_hook():
        if "hook" in state:
            return state["hook"]
        so = "/opt/axon/libaxon_pjrt.so"
        if not os.path.exists(so):
            return None
        lib = ctypes.CDLL(so)
        if not hasattr(lib, "axon_start_nrt_profile"):
            return None
        lib.axon_start_nrt_profile.argtypes = [
            ctypes.POINTER(ctypes.c_int64), ctypes.c_size_t]
        lib.axon_start_nrt_profile.restype = ctypes.c_int64
        lib.axon_stop_nrt_profile.argtypes = [ctypes.c_char_p]
        lib.axon_stop_nrt_profile.restype = ctypes.c_int64

        @contextlib.contextmanager
        def _hook(output_dir, device_ids):
            import jax
            jax.devices()
            if device_ids:
                ids = (ctypes.c_int64 * len(device_ids))(*device_ids)
                rc = lib.axon_start_nrt_profile(ids, len(device_ids))
            else:
                rc = lib.axon_start_nrt_profile(None, 0)
            if rc != 0:
                raise RuntimeError(f"axon_start_nrt_profile rc={rc}")
            try:
                yield
            finally:
                n = lib.axon_stop_nrt_profile(str(output_dir).encode())
                print(f"ntff profile: {n} file(s) written to {output_dir}")

        state["hook"] = _hook
        return _hook

    import antenv
    mod = types.ModuleType("antenv.axon_hooks")
    mod.set_axon_ntff_profile_hook = set_axon_ntff_profile_hook
    mod.get_axon_ntff_profile_hook = get_axon_ntff_profile_hook
    sys.modules["antenv.axon_hooks"] = mod
    antenv.axon_hooks = mod


def _build_program():
    nc = bacc.Bacc("TRN2", target_bir_lowering=False, debug=False)

    xT = nc.dram_tensor("xT", [C, S], BF16, kind="ExternalInput")            # [c, s]
    w0 = nc.dram_tensor("w0", [C, 2 * P], BF16, kind="ExternalInput")        # q-pair0 | k-pair0 cols
    wv = nc.dram_tensor("wv", [C, CPH], BF16, kind="ExternalInput")          # v cols
    wr = nc.dram_tensor("wr", [C, 6 * P], BF16, kind="ExternalInput")        # q-pairs123 | k-pairs123
    qkb = nc.dram_tensor("qkb", [P, 8], F32, kind="ExternalInput")           # q,k bias, partition-major
    bvb = nc.dram_tensor("bvb", [P, CPH], F32, kind="ExternalInput")         # v bias bcast over partitions
    pwT = nc.dram_tensor("pwT", [CPH, C], BF16, kind="ExternalInput")        # [ci, co]
    pbb = nc.dram_tensor("pbb", [P, C], F32, kind="ExternalInput")           # proj bias bcast (zeros on hg=1)
    dmask = nc.dram_tensor("dmask", [P, 4, NQ], BF16, kind="ExternalInput")  # causal 0/1 diag-block mask
    out = nc.dram_tensor("out", [S, C], BF16, kind="ExternalOutput")

    xT_r = xT.rearrange("(o p) s -> p o s", p=P)
    w0_r = w0.rearrange("(o p) m -> p o m", p=P)
    wv_r = wv.rearrange("(o p) m -> p o m", p=P)
    wr_r = wr.rearrange("(o p) m -> p o m", p=P)
    pwT_r = pwT.rearrange("(o p) m -> p o m", p=P)

    with tile.TileContext(nc) as tc:
        with (
            tc.tile_pool(name="const", bufs=1) as const,
            tc.tile_pool(name="work", bufs=4) as work,
            tc.tile_pool(name="psg", bufs=2, space="PSUM") as psum_gen,
            tc.tile_pool(name="pss", bufs=1, space="PSUM") as psum_sc,
            tc.tile_pool(name="psa", bufs=2, space="PSUM") as psum_av,
            tc.tile_pool(name="dram", bufs=4, space="DRAM") as dram,
        ):
            # ---- PE warmup: dummy matmuls on a zeroed tile keep the PE
            # busy through the HAM activity window while the first DMA
            # wave lands, so real matmuls start at 2.4 GHz ----
            zt = const.tile([P, NQ], BF16, tag="zt", name="zt")
            nc.vector.memset(zt, 0.0)
            for w in range(26):
                ps_w = psum_gen.tile([P, NQ], F32, tag="gen", name=f"warm_{w}")
                nc.tensor.matmul(ps_w, lhsT=zt[:, 0:P], rhs=zt, start=True, stop=True)

            # ---- persistent SBUF + input DMAs (ordered by first use;
            # big per-partition lines for DMA-engine throughput) ----
            qkb_sb = const.tile([P, 8], F32, tag="qkb", name="qkb_sb")
            nc.sync.dma_start(out=qkb_sb, in_=qkb[:, :])
            w0_sb = const.tile([P, CO, 2 * P], BF16, tag="w0", name="w0_sb")
            for o in range(CO):
                nc.sync.dma_start(out=w0_sb[:, o, :], in_=w0_r[:, o, :])
            xT_sb = const.tile([P, CO, S], BF16, tag="xT", name="xT_sb")
            for o in range(CO):   # x first half (qk chunks sq 0-1)
                nc.sync.dma_start(out=xT_sb[:, o, 0:2 * NQ], in_=xT_r[:, o, 0:2 * NQ])
            for o in range(CO):   # x second half
                nc.sync.dma_start(out=xT_sb[:, o, 2 * NQ:S], in_=xT_r[:, o, 2 * NQ:S])
            wv_sb = const.tile([P, CO, CPH], BF16, tag="wv", name="wv_sb")
            for o in range(CO):
                nc.sync.dma_start(out=wv_sb[:, o, :], in_=wv_r[:, o, :])
            dm_sb = const.tile([P, 4, NQ], BF16, tag="dmask", name="dm_sb")
            nc.sync.dma_start(out=dm_sb, in_=dmask[:, :, :])
            bvb_sb = const.tile([P, CPH], F32, tag="bvb", name="bvb_sb")
            nc.sync.dma_start(out=bvb_sb, in_=bvb[:, :])
            wr_sb = const.tile([P, CO, 6 * P], BF16, tag="wr", name="wr_sb")
            for o in range(CO):
                nc.sync.dma_start(out=wr_sb[:, o, :], in_=wr_r[:, o, :])
            pwT_sb = const.tile([P, CPH // P, C], BF16, tag="pwT", name="pwT_sb")
            for o in range(CPH // P):
                nc.sync.dma_start(out=pwT_sb[:, o, :], in_=pwT_r[:, o, :])
            pbb_sb = const.tile([P, C], F32, tag="pbb", name="pbb_sb")
            nc.sync.dma_start(out=pbb_sb, in_=pbb[:, :])

            # per-head-pair persistent tensors
            qT_sb = [const.tile([P, S], BF16, tag=f"qT{p}", name=f"qT_sb{p}") for p in range(4)]
            kT_sb = [const.tile([P, S], BF16, tag=f"kT{p}", name=f"kT_sb{p}") for p in range(4)]
            # v: [s-part, kb, pair, parity, d+ones]
            v_sb = const.tile([P, NKB, 4, 2, D + 1], BF16, tag="v", name="v_sb")
            aT_sb = [const.tile([P, S], BF16, tag=f"aT{p}", name=f"aT_sb{p}") for p in range(4)]
            nc.vector.memset(v_sb[:, :, :, :, D:D + 1], 1.0)

            def qk_w(co, kc):
                """stationary weight slice for qkv c'-tile co."""
                if co == 0:
                    return w0_sb[:, kc, 0:P]
                if co == 4:
                    return w0_sb[:, kc, P:2 * P]
                if co < 4:
                    return wr_sb[:, kc, (co - 1) * P:co * P]
                return wr_sb[:, kc, (3 + co - 5) * P:(3 + co - 4) * P]

            def stage1_qk(co, sq, ps_holder=None, kcs=range(CO)):
                """c'-tile co of qkvT (co 0..3 -> qT pair, 4..7 -> kT pair).
                With ps_holder/kcs the contraction can be split into two
                fill-sized halves sharing one PSUM accumulation."""
                dst = qT_sb[co] if co < 4 else kT_sb[co - 4]
                if ps_holder is None or not ps_holder:
                    ps = psum_gen.tile([P, NQ], F32, tag="gen",
                                       name=f"ps_qk_{co}_{sq}")
                    if ps_holder is not None:
                        ps_holder.append(ps)
                else:
                    ps = ps_holder[0]
                for kc in kcs:
                    nc.tensor.matmul(
                        ps,
                        lhsT=qk_w(co, kc),
                        rhs=xT_sb[:, kc, sq * NQ:(sq + 1) * NQ],
                        start=(kc == 0), stop=(kc == CO - 1),
                    )
                if kcs[-1] == CO - 1:
                    nc.vector.tensor_scalar_add(
                        out=dst[:, sq * NQ:(sq + 1) * NQ], in0=ps,
                        scalar1=qkb_sb[:, co:co + 1],
                    )

            def qk_halves(co, sq):
                holder = []
                a = lambda: stage1_qk(co, sq, holder, range(0, CO // 2))
                b = lambda: stage1_qk(co, sq, holder, range(CO // 2, CO))
                return a, b

            def stage1_v(st):
                ps = psum_gen.tile([P, CPH], F32, tag="gen", name=f"ps_v_{st}")
                for kc in range(CO):
                    nc.tensor.matmul(
                        ps,
                        lhsT=xT_sb[:, kc, st * P:(st + 1) * P],
                        rhs=wv_sb[:, kc, :],
                        start=(kc == 0), stop=(kc == CO - 1),
                    )
                nc.vector.tensor_add(
                    out=v_sb[:, st, :, :, 0:D],
                    in0=ps.rearrange("q (a b c) -> q a b c", a=4, b=2),
                    in1=bvb_sb.rearrange("q (a b c) -> q a b c", a=4, b=2),
                )

            st3q = [(st, c2) for st in range(NKB) for c2 in range(2)]
            st3_next = [0]

            def stage3_chunk():
                if st3_next[0] >= len(st3q):
                    return
                st, c2 = st3q[st3_next[0]]
                st3_next[0] += 1
                ps = psum_gen.tile([P, NQ], F32, tag="gen", name=f"ps_o_{st}_{c2}")
                for o in range(4):
                    nc.tensor.matmul(
                        ps,
                        lhsT=aT_sb[o][:, st * P:(st + 1) * P],
                        rhs=pwT_sb[:, o, c2 * NQ:(c2 + 1) * NQ],
                        start=(o == 0), stop=(o == 3),
                    )
                ot = work.tile([P, NQ], BF16, tag="out", name=f"ot_{st}_{c2}")
                nc.vector.tensor_add(out=ot, in0=ps, in1=pbb_sb[:, c2 * NQ:(c2 + 1) * NQ])
                nc.sync.dma_start(
                    out=out[st * P:(st + 1) * P, c2 * NQ:(c2 + 1) * NQ], in_=ot,
                )

            # The normalize runs in three phases deferred across group slots
            # so no DVE instruction ever waits on a DMA at the head of the
            # DVE queue (that would block the causal-mask muls behind it):
            #   a: copy avs out of PSUM (frees banks) + launch the r4
            #      scatter DMA of the ones-row sums
            #   b (1 slot later): reciprocal (r4 has landed) + launch the
            #      DRAM-bounce broadcast of 1/den
            #   c (3 slots later): scale the av rows into the stage-3 input
            def normalize_a(pr, q0, avs):
                st = []
                for par in range(2):
                    av_sb = work.tile([D + 1, NQ], F32, tag=f"avs{par}",
                                      name=f"avs_{pr}_{q0}_{par}")
                    nc.vector.tensor_copy(out=av_sb, in_=avs[par])
                    r4 = work.tile([P, 4], F32, tag=f"r4{par}",
                                   name=f"r4_{pr}_{q0}_{par}")
                    nc.sync.dma_start(out=r4, in_=av_sb[D:D + 1, :])
                    st.append([av_sb, r4, None])
                return st

            def normalize_b(pr, q0, st):
                for par in range(2):
                    av_sb, r4, _ = st[par]
                    nc.vector.reciprocal(out=r4, in_=r4)
                    rdr = dram.tile([NQ], F32, tag=f"rdr{par}",
                                    name=f"rdr_{pr}_{q0}_{par}")
                    nc.sync.dma_start(out=rdr[:], in_=r4)
                    bcs = work.tile([D, NQ], F32, tag=f"bcs{par}",
                                    name=f"bcs_{pr}_{q0}_{par}")
                    rdr_bcast = bass.AP(
                        tensor=rdr.tensor, offset=rdr.offset,
                        ap=[[0, D], rdr.ap[0]],
                    )
                    nc.sync.dma_start(out=bcs, in_=rdr_bcast)
                    st[par][2] = bcs

            def normalize_c(pr, q0, st):
                qs = slice(q0 * NQ, (q0 + 1) * NQ)
                for par in range(2):
                    av_sb, _, bcs = st[par]
                    nc.vector.tensor_mul(
                        out=aT_sb[pr][par * D:(par + 1) * D, qs],
                        in0=av_sb[0:D, :], in1=bcs,
                    )

            # attention pipeline state, carried ACROSS pairs so pair
            # boundaries stay software-pipelined too
            pend = [None]
            defer = []     # [slots_remaining, closure] for deferred muls

            def flush_pend():
                if pend[0] is not None:
                    pend[0]()
                    pend[0] = None

            def tick_defer(force=False):
                for e in defer:
                    e[0] -= 1
                while defer and (force or defer[0][0] <= 0):
                    defer.pop(0)[1]()

            def attention_pair(pr, fills):
                for q0 in range(NSQ):
                    ngrp = 2 * (q0 + 1)          # groups of 2 k-blocks
                    avs = [psum_av.tile([D + 1, NQ], F32, tag="av",
                                        name=f"av_{pr}_{q0}_{par}") for par in range(2)]
                    # for q0>0, emit the 2 diagonal groups FIRST with streams
                    # sliced to the causally-valid q columns [128r:512); the
                    # first emitted block (r=0) is full width so the PSUM
                    # accumulation start covers all columns, and the last
                    # emitted (full-width off-diagonal) group carries stop.
                    # q0==0 keeps full width: its PSUM slots may never have
                    # been written, and sliced scores would leave unbounded
                    # stale data under the exp.
                    order = list(range(ngrp))
                    fl = list(fills.get(q0, []))
                    for n_em, g in enumerate(order):
                        diag = g >= ngrp - 2
                        r0 = (g - (ngrp - 2)) * 2 if diag else 0
                        sliced = diag and q0 > 0
                        ps = psum_sc.tile([P, 2, 2, NQ], F32, tag="sc",
                                          name=f"sc_{pr}_{q0}_{g}")
                        # alternate parity on consecutive matmuls: disjoint
                        # PE row groups run concurrently (row tiling)
                        for i in range(2):
                            kb = 2 * g + i
                            lo = (r0 + i) * P if sliced else 0
                            for par in range(2):
                                base = par * D
                                nc.tensor.matmul(
                                    ps[:, par, i, lo:],
                                    lhsT=kT_sb[pr][base:base + D, kb * P:(kb + 1) * P],
                                    rhs=qT_sb[pr][base:base + D,
                                                  q0 * NQ + lo:(q0 + 1) * NQ],
                                    start=True, stop=True,
                                )
                        lo0 = r0 * P if sliced else 0
                        pt = work.tile([P, 2, 2, NQ], BF16, tag="pt", bufs=2,
                                       name=f"pt_{pr}_{q0}_{g}")
                        nc.scalar.activation(out=pt[:, :, :, lo0:],
                                             in_=ps[:, :, :, lo0:],
                                             func=EXP, scale=0.125)
                        if diag:             # diagonal groups need causal mask
                            for par in range(2):
                                nc.vector.tensor_mul(out=pt[:, par, :, lo0:],
                                                     in0=pt[:, par, :, lo0:],
                                                     in1=dm_sb[:, r0:r0 + 2, lo0:])
                        if fl:
                            f = fl.pop(0)
                            if f is not None:
                                f()
                        flush_pend()
                        tick_defer()

                        def mk_av(g=g, n_em=n_em, pt=pt, sliced=sliced, r0=r0,
                                  q0=q0, avs=avs, ngrp=ngrp):
                            def em():
                                for i in range(2):
                                    kb = 2 * g + i
                                    lo = (r0 + i) * P if sliced else 0
                                    for par in range(2):
                                        nc.tensor.matmul(
                                            avs[par][:, lo:],
                                            lhsT=v_sb[:, kb, pr, par, :],
                                            rhs=pt[:, par, i, lo:],
                                            start=(n_em == 0 and i == 0),
                                            stop=(n_em == ngrp - 1 and i == 1),
                                        )
                                if n_em == ngrp - 1:
                                    st = normalize_a(pr, q0, avs)
                                    defer.append(
                                        [1, lambda: normalize_b(pr, q0, st)])
                                    defer.append(
                                        [3, lambda: normalize_c(pr, q0, st)])
                            return em
                        pend[0] = mk_av()
                    for f in fl:      # leftover fills of this q0
                        if f is not None:
                            f()

            # ---- fill schedule: independent matmul chunks slotted between
            # attention groups so the PE has work while ACT runs the exps.
            # v chunks are locked to pair 0's q0 (AV needs them); qk chunks
            # for pair pr+1 fill pair pr; stage-3 chunks (lagged one q0 for
            # the aT dependency) fill pair 3, remainder after.
            def v2(a):
                return lambda: (stage1_v(a), stage1_v(a + 1))

            def qk(co, sq):
                return lambda: stage1_qk(co, sq)

            def qkh(lst):
                """flatten [(co, sq), ...] into alternating A/B half fills"""
                outl = []
                for co, sq in lst:
                    a, b = qk_halves(co, sq)
                    outl += [a, b]
                return outl

            s3 = stage3_chunk
            fills = [
                {0: [v2(0), v2(2)],
                 1: [v2(4), v2(6)] + qkh([(1, 0)]),
                 2: [v2(8), v2(10)] + qkh([(5, 0), (1, 1)]),
                 3: [v2(12), v2(14)] + qkh([(5, 1), (1, 2), (5, 2)])
                    + [qk(1, 3), qk(5, 3)]},
                {0: qkh([(2, 0)]),
                 1: qkh([(6, 0), (2, 1)]),
                 2: qkh([(6, 1), (2, 2)]) + [None, None],
                 3: qkh([(6, 2), (2, 3), (6, 3)]) + [None, None]},
                {0: qkh([(3, 0)]),
                 1: qkh([(7, 0), (3, 1)]),
                 2: qkh([(7, 1), (3, 2)]) + [None, None],
                 3: qkh([(7, 2), (3, 3), (7, 3)]) + [None, None]},
                {1: [None, None, None, s3],
                 2: [None, None, s3, s3, s3, s3],
                 3: [None, None, s3, s3, s3, s3, s3, s3]},
            ]

            for sq in range(NSQ):
                stage1_qk(0, sq)        # qT pair 0
                stage1_qk(4, sq)        # kT pair 0
            for pr in range(4):
                attention_pair(pr, fills[pr])
            flush_pend()                # AV of the final group + normalize_a
            for _ in range(4):          # cover the bounce latency of the
                stage3_chunk()          # last normalize before its muls
                stage3_chunk()
                tick_defer()
            tick_defer(force=True)
            while st3_next[0] < len(st3q):
                stage3_chunk()

    nc.compile()
    return nc


def _get_nc():
    if not _NC_CACHE:
        _NC_CACHE.append(_build_program())
    return _NC_CACHE[0]


def _make_in_maps(x, qkv_w, qkv_b, proj_w, proj_b):
    x = np.asarray(x, np.float32)
    qkv_w = np.asarray(qkv_w, np.float32)
    qkv_b = np.asarray(qkv_b, np.float32)
    proj_w = np.asarray(proj_w, np.float32)
    proj_b = np.asarray(proj_b, np.float32)

    # causal mask for the 4 diagonal 128x512 blocks of a q-chunk (k <= q)
    kk = np.arange(4)[None, :, None] * P + np.arange(P)[:, None, None]
    qq = np.arange(NQ)[None, None, :]
    dmask = (kk <= qq).astype(BF)

    in_maps = []
    for core in range(8):
        b, hg = core // 2, core % 2
        rows = slice(hg * CPH, (hg + 1) * CPH)
        wq = qkv_w[0 * C:][rows].T     # [1024, 512] columns = q channels
        wk = qkv_w[1 * C:][rows].T
        wvv = qkv_w[2 * C:][rows].T
        bq = qkv_b[0 * C:][rows]
        bk = qkv_b[1 * C:][rows]
        bv = qkv_b[2 * C:][rows]
        in_maps.append({
            "xT": np.ascontiguousarray(x[b].T).astype(BF),
            "w0": np.ascontiguousarray(
                np.concatenate([wq[:, 0:P], wk[:, 0:P]], axis=1)).astype(BF),
            "wv": np.ascontiguousarray(wvv).astype(BF),
            "wr": np.ascontiguousarray(
                np.concatenate([wq[:, P:], wk[:, P:]], axis=1)).astype(BF),
            "qkb": np.ascontiguousarray(
                np.concatenate([bq, bk]).reshape(8, P).T).astype(np.float32),
            "bvb": np.ascontiguousarray(np.tile(bv[None, :], (P, 1))).astype(np.float32),
            "pwT": np.ascontiguousarray(proj_w[:, rows].T).astype(BF),
            "pbb": (np.tile(proj_b[None, :], (P, 1)).astype(np.float32)
                    if hg == 0 else np.zeros((P, C), np.float32)),
            "dmask": dmask,
        })
    return in_maps


def kernel(x, qkv_w, qkv_b, proj_w, proj_b, _trace=False):
    global LAST_RESULTS
    _ensure_axon_hooks()
    in_maps = _make_in_maps(x, qkv_w, qkv_b, proj_w, proj_b)
    nc = _get_nc()
    res = run_bass_kernel_spmd(nc, in_maps, core_ids=list(range(8)), trace=_trace)
    LAST_RESULTS = res
    out = np.empty((B, S, C), np.float32)
    for b in range(B):
        out[b] = (res.results[2 * b]["out"].astype(np.float32)
                  + res.results[2 * b + 1]["out"].astype(np.float32))
    return out


# revision 38
# speedup vs baseline: 1.0548x; 1.0548x over previous
"""Causal self-attention Trainium2 Bass kernel.

Problem: B=4, S=2048, C=1024, H=16 heads, D=64 head_dim.
  qkv = x @ qkv_w.T + qkv_b ; per-head causal softmax attention ; out = attn @ proj_w.T + proj_b

Sharding (8 cores): core = 2*b + hg  (data parallel over batch b=0..3,
tensor parallel over 2 head-groups of 8 heads).  Each core computes
q/k/v for its 8 heads over the full sequence, does causal attention
locally, and computes a partial output projection (contraction over its
512 channels).  Host sums the two partials per batch.

Device layout choices (all matmuls bf16 inputs, fp32 PSUM accumulate):
  - qkvT produced in transposed [c', s] orientation directly from the
    projection (lhsT=wT tile, rhs=xT tile), so per-head qT/kT tiles
    [d=64, s] are ready for the scores matmul with zero transposes.
  - scores computed transposed: sT[k,q] = kT.T @ qT (contraction d on
    partitions).  The two heads of a pair run as concurrent row-tiled
    matmuls (PE rows 0-63 / 64-127).  Softmax sums over k (partition
    dim) come free from a ones-column appended to v in the AV matmul.
    No max-subtraction (init scale 0.02 keeps |scores| small).
  - v produced in [s, c'] orientation (lhsT=xT tile, rhs=wv tile) which
    is exactly the AV lhsT layout.
  - causal masking: full 128x512 score blocks beyond the diagonal are
    skipped; diagonal blocks are stream-sliced to the valid q columns
    and masked after exp.
  - attention is software-pipelined: the AV matmuls of group g are
    emitted one group late so the PE never sits behind the exp (ACT) of
    its own group; independent stage-1/stage-3 matmul chunks are
    slotted between groups to cover the ACT latency.
"""

import numpy as np
import ml_dtypes

import concourse.bacc as bacc
import concourse.bass as bass
import concourse.mybir as mybir
import concourse.tile as tile
from concourse.bass_utils import run_bass_kernel_spmd

BF = ml_dtypes.bfloat16
F32 = mybir.dt.float32
BF16 = mybir.dt.bfloat16
EXP = mybir.ActivationFunctionType.Exp

B, S, C = 4, 2048, 1024
H, D = 16, 64
P = 128
NQ = 512            # q-chunk (psum bank free size)
NSQ = S // NQ       # 4 q-chunks
NKB = S // P        # 16 k-blocks
CO = C // P         # 8 contraction tiles for stage 1
CPH = 512           # channels per head-group (8 heads * 64)

LAST_RESULTS = None
_NC_CACHE = []


def _ensure_axon_hooks():
    """Provide antenv.axon_hooks (NTFF profile hook) when the image lacks it.

    concourse.bass_utils imports it unconditionally on the trace path; this
    container's antenv has no axon_hooks module, but the axon PJRT .so does
    export the profiling C ABI.  Recreates the slim ctypes hook from
    trn_boot._ntff_profile_via_ctypes.  Also stubs out the S3 artifact
    upload (no credentials in-container).
    """
    import sys
    import types
    import contextlib
    import ctypes
    import os

    from concourse import bass_utils as _bu
    _bu.upload_artifacts = lambda tmpdir: str(tmpdir)

    try:
        import antenv.axon_hooks  # noqa: F401
        return
    except ImportError:
        pass

    state = {}

    def set_axon_ntff_profile_hook(hook):
        state["hook"] = hook

    def get_axon_ntff_profile_hook():
        if "hook" in state:
            return state["hook"]
        so = "/opt/axon/libaxon_pjrt.so"
        if not os.path.exists(so):
            return None
        lib = ctypes.CDLL(so)
        if not hasattr(lib, "axon_start_nrt_profile"):
            return None
        lib.axon_start_nrt_profile.argtypes = [
            ctypes.POINTER(ctypes.c_int64), ctypes.c_size_t]
        lib.axon_start_nrt_profile.restype = ctypes.c_int64
        lib.axon_stop_nrt_profile.argtypes = [ctypes.c_char_p]
        lib.axon_stop_nrt_profile.restype = ctypes.c_int64

        @contextlib.contextmanager
        def _hook(output_dir, device_ids):
            import jax
            jax.devices()
            if device_ids:
                ids = (ctypes.c_int64 * len(device_ids))(*device_ids)
                rc = lib.axon_start_nrt_profile(ids, len(device_ids))
            else:
                rc = lib.axon_start_nrt_profile(None, 0)
            if rc != 0:
                raise RuntimeError(f"axon_start_nrt_profile rc={rc}")
            try:
                yield
            finally:
                n = lib.axon_stop_nrt_profile(str(output_dir).encode())
                print(f"ntff profile: {n} file(s) written to {output_dir}")

        state["hook"] = _hook
        return _hook

    import antenv
    mod = types.ModuleType("antenv.axon_hooks")
    mod.set_axon_ntff_profile_hook = set_axon_ntff_profile_hook
    mod.get_axon_ntff_profile_hook = get_axon_ntff_profile_hook
    sys.modules["antenv.axon_hooks"] = mod
    antenv.axon_hooks = mod


def _build_program():
    nc = bacc.Bacc("TRN2", target_bir_lowering=False, debug=False)

    xT = nc.dram_tensor("xT", [C, S], BF16, kind="ExternalInput")            # [c, s]
    w0 = nc.dram_tensor("w0", [C, 2 * P], BF16, kind="ExternalInput")        # q-pair0 | k-pair0 cols
    wv = nc.dram_tensor("wv", [C, CPH], BF16, kind="ExternalInput")          # v cols
    wr = nc.dram_tensor("wr", [C, 6 * P], BF16, kind="ExternalInput")        # q-pairs123 | k-pairs123
    qkb = nc.dram_tensor("qkb", [P, 8], F32, kind="ExternalInput")           # q,k bias, partition-major
    bvb = nc.dram_tensor("bvb", [P, CPH], F32, kind="ExternalInput")         # v bias bcast over partitions
    pwT = nc.dram_tensor("pwT", [CPH, C], BF16, kind="ExternalInput")        # [ci, co]
    pbb = nc.dram_tensor("pbb", [P, C], F32, kind="ExternalInput")           # proj bias bcast (zeros on hg=1)
    dmask = nc.dram_tensor("dmask", [P, 4, NQ], BF16, kind="ExternalInput")  # causal 0/1 diag-block mask
    out = nc.dram_tensor("out", [S, C], BF16, kind="ExternalOutput")

    xT_r = xT.rearrange("(o p) s -> p o s", p=P)
    w0_r = w0.rearrange("(o p) m -> p o m", p=P)
    wv_r = wv.rearrange("(o p) m -> p o m", p=P)
    wr_r = wr.rearrange("(o p) m -> p o m", p=P)
    pwT_r = pwT.rearrange("(o p) m -> p o m", p=P)

    with tile.TileContext(nc) as tc:
        with (
            tc.tile_pool(name="const", bufs=1) as const,
            tc.tile_pool(name="work", bufs=4) as work,
            tc.tile_pool(name="psg", bufs=2, space="PSUM") as psum_gen,
            tc.tile_pool(name="pss", bufs=1, space="PSUM") as psum_sc,
            tc.tile_pool(name="psa", bufs=2, space="PSUM") as psum_av,
            tc.tile_pool(name="dram", bufs=4, space="DRAM") as dram,
        ):
            # ---- PE warmup: dummy matmuls on a zeroed tile keep the PE
            # busy through the HAM activity window while the first DMA
            # wave lands, so real matmuls start at 2.4 GHz ----
            zt = const.tile([P, NQ], BF16, tag="zt", name="zt")
            nc.vector.memset(zt, 0.0)
            for w in range(26):
                ps_w = psum_gen.tile([P, NQ], F32, tag="gen", name=f"warm_{w}")
                nc.tensor.matmul(ps_w, lhsT=zt[:, 0:P], rhs=zt, start=True, stop=True)
            # zero the scores psum slot so sliced q0=0 diagonal groups never
            # exp() unwritten PSUM
            ps_s = psum_sc.tile([P, 2, 2, NQ], F32, tag="sc", name="warm_sc")
            for j in range(4):
                nc.tensor.matmul(ps_s[:, j // 2, j % 2, :], lhsT=zt[:, 0:P],
                                 rhs=zt, start=True, stop=True)

            # ---- persistent SBUF + input DMAs (ordered by first use;
            # big per-partition lines for DMA-engine throughput) ----
            qkb_sb = const.tile([P, 8], F32, tag="qkb", name="qkb_sb")
            nc.sync.dma_start(out=qkb_sb, in_=qkb[:, :])
            w0_sb = const.tile([P, CO, 2 * P], BF16, tag="w0", name="w0_sb")
            for o in range(CO):
                nc.sync.dma_start(out=w0_sb[:, o, :], in_=w0_r[:, o, :])
            xT_sb = const.tile([P, CO, S], BF16, tag="xT", name="xT_sb")
            for o in range(CO):   # x first half (qk chunks sq 0-1)
                nc.sync.dma_start(out=xT_sb[:, o, 0:2 * NQ], in_=xT_r[:, o, 0:2 * NQ])
            for o in range(CO):   # x second half
                nc.sync.dma_start(out=xT_sb[:, o, 2 * NQ:S], in_=xT_r[:, o, 2 * NQ:S])
            wv_sb = const.tile([P, CO, CPH], BF16, tag="wv", name="wv_sb")
            for o in range(CO):
                nc.sync.dma_start(out=wv_sb[:, o, :], in_=wv_r[:, o, :])
            dm_sb = const.tile([P, 4, NQ], BF16, tag="dmask", name="dm_sb")
            nc.sync.dma_start(out=dm_sb, in_=dmask[:, :, :])
            bvb_sb = const.tile([P, CPH], F32, tag="bvb", name="bvb_sb")
            nc.sync.dma_start(out=bvb_sb, in_=bvb[:, :])
            wr_sb = const.tile([P, CO, 6 * P], BF16, tag="wr", name="wr_sb")
            for o in range(CO):
                nc.sync.dma_start(out=wr_sb[:, o, :], in_=wr_r[:, o, :])
            pwT_sb = const.tile([P, CPH // P, C], BF16, tag="pwT", name="pwT_sb")
            for o in range(CPH // P):
                nc.sync.dma_start(out=pwT_sb[:, o, :], in_=pwT_r[:, o, :])
            pbb_sb = const.tile([P, C], F32, tag="pbb", name="pbb_sb")
            nc.sync.dma_start(out=pbb_sb, in_=pbb[:, :])

            # per-head-pair persistent tensors
            qT_sb = [const.tile([P, S], BF16, tag=f"qT{p}", name=f"qT_sb{p}") for p in range(4)]
            kT_sb = [const.tile([P, S], BF16, tag=f"kT{p}", name=f"kT_sb{p}") for p in range(4)]
            # v: [s-part, kb, pair, parity, d+ones]
            v_sb = const.tile([P, NKB, 4, 2, D + 1], BF16, tag="v", name="v_sb")
            aT_sb = [const.tile([P, S], BF16, tag=f"aT{p}", name=f"aT_sb{p}") for p in range(4)]
            nc.vector.memset(v_sb[:, :, :, :, D:D + 1], 1.0)

            def qk_w(co, kc):
                """stationary weight slice for qkv c'-tile co."""
                if co == 0:
                    return w0_sb[:, kc, 0:P]
                if co == 4:
                    return w0_sb[:, kc, P:2 * P]
                if co < 4:
                    return wr_sb[:, kc, (co - 1) * P:co * P]
                return wr_sb[:, kc, (3 + co - 5) * P:(3 + co - 4) * P]

            def stage1_qk(co, sq, ps_holder=None, kcs=range(CO)):
                """c'-tile co of qkvT (co 0..3 -> qT pair, 4..7 -> kT pair).
                With ps_holder/kcs the contraction can be split into two
                fill-sized halves sharing one PSUM accumulation."""
                dst = qT_sb[co] if co < 4 else kT_sb[co - 4]
                if ps_holder is None or not ps_holder:
                    ps = psum_gen.tile([P, NQ], F32, tag="gen",
                                       name=f"ps_qk_{co}_{sq}")
                    if ps_holder is not None:
                        ps_holder.append(ps)
                else:
                    ps = ps_holder[0]
                for kc in kcs:
                    nc.tensor.matmul(
                        ps,
                        lhsT=qk_w(co, kc),
                        rhs=xT_sb[:, kc, sq * NQ:(sq + 1) * NQ],
                        start=(kc == 0), stop=(kc == CO - 1),
                    )
                if kcs[-1] == CO - 1:
                    nc.vector.tensor_scalar_add(
                        out=dst[:, sq * NQ:(sq + 1) * NQ], in0=ps,
                        scalar1=qkb_sb[:, co:co + 1],
                    )

            def qk_halves(co, sq):
                holder = []
                a = lambda: stage1_qk(co, sq, holder, range(0, CO // 2))
                b = lambda: stage1_qk(co, sq, holder, range(CO // 2, CO))
                return a, b

            def stage1_v(st):
                ps = psum_gen.tile([P, CPH], F32, tag="gen", name=f"ps_v_{st}")
                for kc in range(CO):
                    nc.tensor.matmul(
                        ps,
                        lhsT=xT_sb[:, kc, st * P:(st + 1) * P],
                        rhs=wv_sb[:, kc, :],
                        start=(kc == 0), stop=(kc == CO - 1),
                    )
                nc.vector.tensor_add(
                    out=v_sb[:, st, :, :, 0:D],
                    in0=ps.rearrange("q (a b c) -> q a b c", a=4, b=2),
                    in1=bvb_sb.rearrange("q (a b c) -> q a b c", a=4, b=2),
                )

            st3q = [(st, c2) for st in range(NKB) for c2 in range(2)]
            st3_next = [0]

            def stage3_chunk():
                if st3_next[0] >= len(st3q):
                    return
                st, c2 = st3q[st3_next[0]]
                st3_next[0] += 1
                ps = psum_gen.tile([P, NQ], F32, tag="gen", name=f"ps_o_{st}_{c2}")
                for o in range(4):
                    nc.tensor.matmul(
                        ps,
                        lhsT=aT_sb[o][:, st * P:(st + 1) * P],
                        rhs=pwT_sb[:, o, c2 * NQ:(c2 + 1) * NQ],
                        start=(o == 0), stop=(o == 3),
                    )
                ot = work.tile([P, NQ], BF16, tag="out", name=f"ot_{st}_{c2}")
                nc.vector.tensor_add(out=ot, in0=ps, in1=pbb_sb[:, c2 * NQ:(c2 + 1) * NQ])
                nc.sync.dma_start(
                    out=out[st * P:(st + 1) * P, c2 * NQ:(c2 + 1) * NQ], in_=ot,
                )

            # The normalize runs in three phases deferred across group slots
            # so no DVE instruction ever waits on a DMA at the head of the
            # DVE queue (that would block the causal-mask muls behind it):
            #   a: copy avs out of PSUM (frees banks) + launch the r4
            #      scatter DMA of the ones-row sums
            #   b (1 slot later): reciprocal (r4 has landed) + launch the
            #      DRAM-bounce broadcast of 1/den
            #   c (3 slots later): scale the av rows into the stage-3 input
            def normalize_a(pr, q0, avs):
                st = []
                for par in range(2):
                    av_sb = work.tile([D + 1, NQ], F32, tag=f"avs{par}",
                                      name=f"avs_{pr}_{q0}_{par}")
                    nc.vector.tensor_copy(out=av_sb, in_=avs[par])
                    r4 = work.tile([P, 4], F32, tag=f"r4{par}",
                                   name=f"r4_{pr}_{q0}_{par}")
                    nc.sync.dma_start(out=r4, in_=av_sb[D:D + 1, :])
                    st.append([av_sb, r4, None])
                return st

            def normalize_b(pr, q0, st):
                for par in range(2):
                    av_sb, r4, _ = st[par]
                    nc.vector.reciprocal(out=r4, in_=r4)
                    rdr = dram.tile([NQ], F32, tag=f"rdr{par}",
                                    name=f"rdr_{pr}_{q0}_{par}")
                    nc.sync.dma_start(out=rdr[:], in_=r4)
                    bcs = work.tile([D, NQ], F32, tag=f"bcs{par}",
                                    name=f"bcs_{pr}_{q0}_{par}")
                    rdr_bcast = bass.AP(
                        tensor=rdr.tensor, offset=rdr.offset,
                        ap=[[0, D], rdr.ap[0]],
                    )
                    nc.sync.dma_start(out=bcs, in_=rdr_bcast)
                    st[par][2] = bcs

            def normalize_c(pr, q0, st):
                qs = slice(q0 * NQ, (q0 + 1) * NQ)
                for par in range(2):
                    av_sb, _, bcs = st[par]
                    nc.vector.tensor_mul(
                        out=aT_sb[pr][par * D:(par + 1) * D, qs],
                        in0=av_sb[0:D, :], in1=bcs,
                    )

            # attention pipeline state, carried ACROSS pairs so pair
            # boundaries stay software-pipelined too
            pend = [None]
            defer = []     # [slots_remaining, closure] for deferred muls

            def flush_pend():
                if pend[0] is not None:
                    pend[0]()
                    pend[0] = None

            def tick_defer(force=False):
                for e in defer:
                    e[0] -= 1
                while defer and (force or defer[0][0] <= 0):
                    defer.pop(0)[1]()

            def attention_pair(pr, fills):
                for q0 in range(NSQ):
                    ngrp = 2 * (q0 + 1)          # groups of 2 k-blocks
                    avs = [psum_av.tile([D + 1, NQ], F32, tag="av",
                                        name=f"av_{pr}_{q0}_{par}") for par in range(2)]
                    # for q0>0, emit the 2 diagonal groups FIRST with streams
                    # sliced to the causally-valid q columns [128r:512); the
                    # first emitted block (r=0) is full width so the PSUM
                    # accumulation start covers all columns, and the last
                    # emitted (full-width off-diagonal) group carries stop.
                    # q0==0 keeps full width: its PSUM slots may never have
                    # been written, and sliced scores would leave unbounded
                    # stale data under the exp.
                    if q0 == 0:
                        order = list(range(ngrp))
                    else:
                        order = [ngrp - 2, ngrp - 1] + list(range(ngrp - 2))
                    fl = list(fills.get(q0, []))
                    for n_em, g in enumerate(order):
                        diag = g >= ngrp - 2
                        r0 = (g - (ngrp - 2)) * 2 if diag else 0
                        sliced = diag
                        ps = psum_sc.tile([P, 2, 2, NQ], F32, tag="sc",
                                          name=f"sc_{pr}_{q0}_{g}")
                        # alternate parity on consecutive matmuls: disjoint
                        # PE row groups run concurrently (row tiling)
                        for i in range(2):
                            kb = 2 * g + i
                            lo = (r0 + i) * P if sliced else 0
                            for par in range(2):
                                base = par * D
                                nc.tensor.matmul(
                                    ps[:, par, i, lo:],
                                    lhsT=kT_sb[pr][base:base + D, kb * P:(kb + 1) * P],
                                    rhs=qT_sb[pr][base:base + D,
                                                  q0 * NQ + lo:(q0 + 1) * NQ],
                                    start=True, stop=True,
                                )
                        lo0 = r0 * P if sliced else 0
                        pt = work.tile([P, 2, 2, NQ], BF16, tag="pt", bufs=2,
                                       name=f"pt_{pr}_{q0}_{g}")
                        nc.scalar.activation(out=pt[:, :, :, lo0:],
                                             in_=ps[:, :, :, lo0:],
                                             func=EXP, scale=0.125)
                        if diag:             # diagonal groups need causal mask
                            for par in range(2):
                                nc.vector.tensor_mul(out=pt[:, par, :, lo0:],
                                                     in0=pt[:, par, :, lo0:],
                                                     in1=dm_sb[:, r0:r0 + 2, lo0:])
                        if fl:
                            f = fl.pop(0)
                            if f is not None:
                                f()
                        flush_pend()
                        tick_defer()

                        def mk_av(g=g, n_em=n_em, pt=pt, sliced=sliced, r0=r0,
                                  q0=q0, avs=avs, ngrp=ngrp):
                            def em():
                                for i in range(2):
                                    kb = 2 * g + i
                                    lo = (r0 + i) * P if sliced else 0
                                    for par in range(2):
                                        nc.tensor.matmul(
                                            avs[par][:, lo:],
                                            lhsT=v_sb[:, kb, pr, par, :],
                                            rhs=pt[:, par, i, lo:],
                                            start=(n_em == 0 and i == 0),
                                            stop=(n_em == ngrp - 1 and i == 1),
                                        )
                                if n_em == ngrp - 1:
                                    st = normalize_a(pr, q0, avs)
                                    defer.append(
                                        [1, lambda: normalize_b(pr, q0, st)])
                                    defer.append(
                                        [3, lambda: normalize_c(pr, q0, st)])
                            return em
                        pend[0] = mk_av()
                    for f in fl:      # leftover fills of this q0
                        if f is not None:
                            f()

            # ---- fill schedule: independent matmul chunks slotted between
            # attention groups so the PE has work while ACT runs the exps.
            # v chunks are locked to pair 0's q0 (AV needs them); qk chunks
            # for pair pr+1 fill pair pr; stage-3 chunks (lagged one q0 for
            # the aT dependency) fill pair 3, remainder after.
            def v2(a):
                return lambda: (stage1_v(a), stage1_v(a + 1))

            def qk(co, sq):
                return lambda: stage1_qk(co, sq)

            def qkh(lst):
                """flatten [(co, sq), ...] into alternating A/B half fills"""
                outl = []
                for co, sq in lst:
                    a, b = qk_halves(co, sq)
                    outl += [a, b]
                return outl

            s3 = stage3_chunk
            fills = [
                {0: [v2(0), v2(2)],
                 1: [v2(4), v2(6)] + qkh([(1, 0)]),
                 2: [v2(8), v2(10)] + qkh([(5, 0), (1, 1)]),
                 3: [v2(12), v2(14)] + qkh([(5, 1), (1, 2), (5, 2)])
                    + [qk(1, 3), qk(5, 3)]},
                {0: qkh([(2, 0)]),
                 1: qkh([(6, 0), (2, 1)]),
                 2: qkh([(6, 1), (2, 2)]) + [None, None],
                 3: qkh([(6, 2), (2, 3), (6, 3)]) + [None, None]},
                {0: qkh([(3, 0)]),
                 1: qkh([(7, 0), (3, 1)]),
                 2: qkh([(7, 1), (3, 2)]) + [None, None],
                 3: qkh([(7, 2), (3, 3), (7, 3)]) + [None, None]},
                {1: [None, None, None, s3],
                 2: [None, None, s3, s3, s3, s3],
                 3: [None, None, s3, s3, s3, s3, s3, s3]},
            ]

            for sq in range(NSQ):
                stage1_qk(0, sq)        # qT pair 0
                stage1_qk(4, sq)        # kT pair 0
            for pr in range(4):
                attention_pair(pr, fills[pr])
            flush_pend()                # AV of the final group + normalize_a
            for _ in range(4):          # cover the bounce latency of the
                stage3_chunk()          # last normalize before its muls
                stage3_chunk()
                tick_defer()
            tick_defer(force=True)
            while st3_next[0] < len(st3q):
                stage3_chunk()

    nc.compile()
    return nc


def _get_nc():
    if not _NC_CACHE:
        _NC_CACHE.append(_build_program())
    return _NC_CACHE[0]


def _make_in_maps(x, qkv_w, qkv_b, proj_w, proj_b):
    x = np.asarray(x, np.float32)
    qkv_w = np.asarray(qkv_w, np.float32)
    qkv_b = np.asarray(qkv_b, np.float32)
    proj_w = np.asarray(proj_w, np.float32)
    proj_b = np.asarray(proj_b, np.float32)

    # causal mask for the 4 diagonal 128x512 blocks of a q-chunk (k <= q)
    kk = np.arange(4)[None, :, None] * P + np.arange(P)[:, None, None]
    qq = np.arange(NQ)[None, None, :]
    dmask = (kk <= qq).astype(BF)

    in_maps = []
    for core in range(8):
        b, hg = core // 2, core % 2
        rows = slice(hg * CPH, (hg + 1) * CPH)
        wq = qkv_w[0 * C:][rows].T     # [1024, 512] columns = q channels
        wk = qkv_w[1 * C:][rows].T
        wvv = qkv_w[2 * C:][rows].T
        bq = qkv_b[0 * C:][rows]
        bk = qkv_b[1 * C:][rows]
        bv = qkv_b[2 * C:][rows]
        in_maps.append({
            "xT": np.ascontiguousarray(x[b].T).astype(BF),
            "w0": np.ascontiguousarray(
                np.concatenate([wq[:, 0:P], wk[:, 0:P]], axis=1)).astype(BF),
            "wv": np.ascontiguousarray(wvv).astype(BF),
            "wr": np.ascontiguousarray(
                np.concatenate([wq[:, P:], wk[:, P:]], axis=1)).astype(BF),
            "qkb": np.ascontiguousarray(
                np.concatenate([bq, bk]).reshape(8, P).T).astype(np.float32),
            "bvb": np.ascontiguousarray(np.tile(bv[None, :], (P, 1))).astype(np.float32),
            "pwT": np.ascontiguousarray(proj_w[:, rows].T).astype(BF),
            "pbb": (np.tile(proj_b[None, :], (P, 1)).astype(np.float32)
                    if hg == 0 else np.zeros((P, C), np.float32)),
            "dmask": dmask,
        })
    return in_maps


def kernel(x, qkv_w, qkv_b, proj_w, proj_b, _trace=False):
    global LAST_RESULTS
    _ensure_axon_hooks()
    in_maps = _make_in_maps(x, qkv_w, qkv_b, proj_w, proj_b)
    nc = _get_nc()
    res = run_bass_kernel_spmd(nc, in_maps, core_ids=list(range(8)), trace=_trace)
    LAST_RESULTS = res
    out = np.empty((B, S, C), np.float32)
    for b in range(B):
        out[b] = (res.results[2 * b]["out"].astype(np.float32)
                  + res.results[2 * b + 1]["out"].astype(np.float32))
    return out
